# revision 29
# baseline (speedup 1.0000x reference)
"""Trainium2 Bass kernel for nn_MixedFeedFoward (DARTS-style mixed-architecture MLP).

Math: out = relu(x @ (m0*w0).T + bm0*b0) @ (m1*w1).T + bm1*b1
The DARTS masks are rank-structured.  With a = softmax(arch_embed),
b = softmax(arch_mlp), EMBED = (512,768,1024), RATIO = (2,3,4):

  s_e[h]     = sum_r b_r * [h < e*r]
  g_j[h]     = sum_{e_idx >= j} a_e * s_e[h]
  c_j        = sum_{e_idx >= j} a_e
  W0eff[h,d] = w0[h,d] * g_{blk(d)}[h]      blk(d): 0 for d<512, 1 for d<768, else 2
  bm0[h]     = g_0[h]
  W1eff[d,h] = w1[d,h] * g_{blk(d)}[h]
  bm1[d]     = c_{blk(d)}

g_j is constant on 256-aligned h segments, so masking reduces to 51 runtime
scalars computed on device from the arch inputs via one tiny matmul against a
constant 0/1 selection table (e9 broadcast to 128 columns makes the matmul
output land on all 128 partitions directly).  The softmax normalizer Z
(col 51, all-ones) is NOT applied on the critical path: w0 is masked with the
UNNORMALIZED gu = Z*g (h comes out scaled by Z), and the w1 mask scalars are
premultiplied by 1/Z^2, so L1 partials land at true scale and the tail
finalize is a plain add+store.  relu commutes with the positive Z scaling, so
numerics match the normalized form to bf16 rounding.

Sharding: data-parallel over the 4096 tokens -> 512 tokens per core; every
core streams the full weights.  PE roofline: 512 matmuls x 216 ns = 110.6 us.
The schedule is supply-shaped: measured per-core in-bound DMA is ~300 GB/s
for the first ~4 MiB (8-core HBM contention at start) and ~400 GB/s after,
while real matmuls demand 290 GB/s — so the opening is data-limited and the
binding bound is  t(first 4 MiB) + (512-32)*216ns + tail  ~=  131 us.

  - q1 (sync HWDGE) carries, in exact k-step consumption order: arch row,
    x0 x1, w0(hg0,pk0), x2 x3, w0(pk1), x4 x5, w0(pk2), x6 x7, w0(pk3),
    w0(hg1), then the steady w1/w0 phase stream.  Output stores also go on
    q1's engine (sync) — it is idle in the tail, unlike Scalar whose queue
    serializes ~0.6 us per DMA trigger behind its activation work.
  - PE warmup (pinned to scheduler priority 0 with its junk memsets so it
    starts at engine boot ~8 us every build): NW1 big junk matmuls bridge
    the HAM cold window, the tiny mask matmul runs when exp(arch) lands
    (~12 us; DMA completion sems add ~2 us over data arrival), then NW2
    64-column junk matmuls (54 ns each, so overshoot is nearly free against
    the free-running +-3.4 us HAM window phase), then real work ~14 us.
  - PACE_* junk matmuls are interleaved into hg0's k-steps, sized to the
    measured supply curve, so the PE never accumulates enough idle density
    to re-trigger the HAM half-clock window (2x cost on the next ~7 us).
Steady state: L0 k-major with 4 concurrent PSUM chains per 512-row h-group
(even h-groups on ps0 banks, odd on ps1); L1 for pr<3 runs pj-major across
ALL 8 output chains (8 PSUM banks, w1 pairs consumed in DMA arrival order);
L1(pr3) runs dt-major so chains end staggered and the store tail pipelines,
with the last dt as two column-half chains (its final store covers 128 KB).
All w0/w1 masks on DVE, emitted in arrival order (ACT masks would queue
behind relu drains, holding w1f bufs and throttling prefetch; GpSimd is 3x
too slow); L1 finalizes are DEFERRED into the next L0's mask emission so
mask production never queues behind 8 back-to-back PSUM-drain adds, and
each freed ps bank is recycled just-in-time.  pr0 finalize (copy+bias) runs
on ACT via AF.Identity.

Remaining known overheads (measured): ~8 us engine boot, ~4 us HAM cold
ramp, ~6 us data-bound opening slack, ~10x ~432 ns periodic PE stalls
(LDWEIGHTS/start-matmuls waiting on just-in-time DVE mask production and
PSUM-bank drain sems -- rebalancing engines (ACT/GpSimd masks, DVE relus)
was tried repeatedly and always shifted, never removed, these waits; the
Tile scheduler's static schedule also swings +-2 us between builds), and
~6 us tail (2 us last add+store+flight+receipt, ~3.5 us NEFF drain
protocol).  fp8 DoubleRow (2x PE) was measured working at 216 ns/instr
but is numerically infeasible: e4m3 quantization is 2.65% RMS per tensor
-> >=3.7% output error vs the 2e-2 gate (bf16 gives 3.3e-3).  Multi-queue
DMA does NOT raise the ~390 GB/s per-core in-bound cap (HBM-per-NC
limit).  Final opening: hg0's mask ops read the mask-scalar PSUM directly
(the gbu copy + derived scalars are emitted after hg0's masks, off the
first-matmul critical path), and NW2 is sized for the COLD tiny-junk rate
so a late HAM fire cannot push real work past data-readiness.  Do NOT
shrink the xf staging tiles into a reused ring: a staging-slot reuse makes
an x-load DMA trigger wait on a cast, and since the sync queue is in-order
that stalls the entire weight stream behind it (measured 165 us).  SBUF is
full: w0f=10 overflows by ~4 KB/partition.  Measured 136.6-138.0 us over
8 runs, one 163.7 us outlier from transient machine noise (was
142.5-144.4).
"""

import os

import numpy as np

import concourse.bass as bass
import concourse.mybir as mybir
from concourse import bacc
from concourse.bass_utils import run_bass_kernel_spmd
from concourse.tile import TileContext

N_CORES = 8
D = 1024          # embed dim
H = 4096          # expansion dim
T = 512           # tokens per core (4096 total / 8 cores)
P = 128
SEG = 256         # h-segment size on which g_j is constant
NSEG = H // SEG   # 16
EMBED = (512, 768, 1024)
RATIO = (2, 3, 4)

F32 = mybir.dt.float32
BF16 = mybir.dt.bfloat16
AF = mybir.ActivationFunctionType
ALU = mybir.AluOpType

NW1 = int(os.environ.get("BASS_NW1", "13"))     # junk MMs before the mask matmul
NW2 = int(os.environ.get("BASS_NW2", "16"))    # junk MMs after it, before real work
# pacing junk inside the supply-starved opening: {after_kstep: count}
PACE_HG0 = {1: int(os.environ.get("BASS_PH0A", "8")),
            3: int(os.environ.get("BASS_PH0B", "3")),
            5: int(os.environ.get("BASS_PH0C", "5"))}
PACE_HG1 = {3: int(os.environ.get("BASS_PH1A", "0"))}
FILL2 = int(os.environ.get("BASS_FILL2", "0"))


def _build_k2() -> np.ndarray:
    """Constant 0/1 selection table: G_flat[col] = sum_i E9[i] * K2[i, col]
    where E9[e*3+r] = exp(ae[e] + am[r]).
    cols 0..47: col = j*16 + seg -> [e_idx >= j] * [seg*SEG < e*r]
    cols 48..50: col = 48 + j   -> [e_idx >= j]   (since sum_r b_r = 1)
    col 51: all ones -> sum(E9), the softmax normalizer Z
    """
    k2 = np.zeros((9, 52), dtype=np.float32)
    for ie, e in enumerate(EMBED):
        for ir, r in enumerate(RATIO):
            i = ie * 3 + ir
            for j in range(3):
                if ie >= j:
                    for seg in range(NSEG):
                        if seg * SEG < e * r:
                            k2[i, j * 16 + seg] = 1.0
                    k2[i, 48 + j] = 1.0
            k2[i, 51] = 1.0
    return k2


_K2 = _build_k2()

# d-block of each 128-wide d-chunk (0..7): [0,512)->0, [512,768)->1, [768,1024)->2
_DBLK = [0, 0, 0, 0, 1, 1, 2, 2]


def _build_nc() -> bass.Bass:
    nc = bacc.Bacc("TRN2", target_bir_lowering=False, debug=False)

    xT_d = nc.dram_tensor("xT", [D, T], F32, kind="ExternalInput")
    w0T_d = nc.dram_tensor("w0T", [D, H], F32, kind="ExternalInput")
    w1T_d = nc.dram_tensor("w1T", [H, D], F32, kind="ExternalInput")
    b0r_d = nc.dram_tensor("b0r", [P, H // P], F32, kind="ExternalInput")
    b1r_d = nc.dram_tensor("b1r", [P, D // P], F32, kind="ExternalInput")
    # arch = [ae9 | am9 | K2] packed in one tensor: a single 216B-row DMA
    arch_d = nc.dram_tensor("arch", [9, 54], F32, kind="ExternalInput")
    out_d = nc.dram_tensor("outT", [D, T], F32, kind="ExternalOutput")

    with TileContext(nc) as tc:
        with (
            tc.tile_pool(name="const", bufs=1) as const,
            tc.tile_pool(name="w0f", bufs=8) as w0f_pool,
            tc.tile_pool(name="xfp", bufs=1) as xf_pool,
            tc.tile_pool(name="w0p", bufs=16) as w0_pool,
            tc.tile_pool(name="w1f", bufs=7) as w1f_pool,
            tc.tile_pool(name="w1p", bufs=6) as w1_pool,
            tc.tile_pool(name="ps0", bufs=4, space="PSUM") as ps0_pool,
            tc.tile_pool(name="ps1", bufs=4, space="PSUM") as ps1_pool,
        ):
            # ---------------- tiny constants (no deps; DVE runs them first) --
            # junk memsets + warmup pinned to scheduler priority 0: the PE
            # warmup must start right at engine boot (~7.8 us) on every build
            junk_w = const.tile([P, 2 * P], BF16, tag="junk_w")
            junk_x = const.tile([P, T], BF16, tag="junk_x")
            with tc.high_priority():
                nc.vector.memset(junk_w[:], 0.0)
                nc.vector.memset(junk_x[:], 0.0)
                ps_w = ps0_pool.tile([P, T], F32, tag="ps0", name="ps_w")
                for i in range(NW1):
                    sl = (i % 2) * P
                    nc.tensor.matmul(
                        ps_w[:], junk_w[:, sl:sl + P], junk_x[:],
                        start=(i == 0), stop=(i == NW1 - 1),
                    )
            ps_w = ps0_pool.tile([P, T], F32, tag="ps0", name="ps_w")
            for i in range(NW1):
                sl = (i % 2) * P
                nc.tensor.matmul(
                    ps_w[:], junk_w[:, sl:sl + P], junk_x[:],
                    start=(i == 0), stop=(i == NW1 - 1),
                )

            ones9 = const.tile([9, P], F32, tag="ones9")
            nc.vector.memset(ones9[:], 1.0)

            # ---------------- arch-weight prep (q1 first: tiny, low latency) -
            arch_sb = const.tile([9, 54], F32, tag="arch_sb")
            nc.sync.dma_start(arch_sb[:], arch_d[:, :])
            k2_sb = arch_sb[:, 2:54]

            # e9 = exp(ae + am) in one ACT op; broadcast along free dim on DVE
            e9 = const.tile([9, 1], F32, tag="e9")
            nc.scalar.activation(e9[:], arch_sb[:, 0:1], AF.Exp, bias=arch_sb[:, 1:2])
            e9r = const.tile([9, P], F32, tag="e9r")
            nc.vector.tensor_scalar(e9r[:], ones9[:], e9[:, 0:1], None, ALU.mult)

            # biases ride the scalar HWDGE queue (triggers issue before exp)
            b0_sb = const.tile([P, H // P], F32, tag="b0_sb")
            nc.scalar.dma_start(b0_sb[:], b0r_d[:, :])
            b1_sb = const.tile([P, D // P], F32, tag="b1_sb")
            nc.scalar.dma_start(b1_sb[:], b1r_d[:, :])

            # ---------------- x + w0(hg0/hg1) on q1 in consumption order -----
            xfs = []
            xt_sb = []
            for k in range(D // P):
                xf = xf_pool.tile([P, T], F32, tag=f"xf{k}", name=f"xf{k}", bufs=1)
                t = const.tile([P, T], BF16, tag=f"xt{k}", name=f"xt{k}")
                xfs.append(xf)
                xt_sb.append(t)

            def load_x(k):
                nc.sync.dma_start(xfs[k][:], xT_d[k * P:(k + 1) * P, :])

            w0f_tiles = {}  # (hg, pk) -> tile

            def load_w0f(hg, pk):
                w0f = w0f_pool.tile([P, 1024], F32, tag="w0f", name="w0f")
                w0f_tiles[(hg, pk)] = w0f
                nc.sync.dma_start(
                    w0f[:].rearrange("p (k h) -> p k h", k=2),
                    w0T_d[
                        2 * pk * P:(2 * pk + 2) * P,
                        hg * 512:(hg + 1) * 512,
                    ].rearrange("(k p) h -> p k h", k=2),
                )

            # q1 order = k-step consumption order of L0(hg0)
            load_x(0)
            load_x(1)
            load_w0f(0, 0)
            load_x(2)
            load_x(3)
            load_w0f(0, 1)
            load_x(4)
            load_x(5)
            load_w0f(0, 2)
            load_x(6)
            load_x(7)
            load_w0f(0, 3)
            for pk in range(4):
                load_w0f(1, pk)

            # x casts: first two chunks on DVE (needed by the first k-steps),
            # the rest on Scalar (queued after exp)
            for k in range(2):
                nc.vector.tensor_copy(xt_sb[k][:], xfs[k][:])
            for k in range(2, D // P):
                nc.scalar.activation(xt_sb[k][:], xfs[k][:], AF.Copy)

            # ---------------- PE: mask matmul, then junk until data -------
            g_ps = ps1_pool.tile([P, T], F32, tag="ps1", name="g_ps")[:, 0:52]
            nc.tensor.matmul(g_ps[:], e9r[:], k2_sb[:], start=True, stop=True)
            ps_w2 = ps0_pool.tile([P, T], F32, tag="ps0", name="ps_w2")
            for i in range(NW2):
                sl = (i % 2) * P
                nc.tensor.matmul(
                    ps_w2[:, 0:64], junk_w[:, sl:sl + P], junk_x[:, 0:64],
                    start=(i == 0), stop=(i == NW2 - 1),
                )

            # gbu = unnormalized mask scalars [128, 52]; col 51 = Z.
            # hg0's mask ops read g_ps (PSUM) directly -- the copy and all
            # derived scalars are emitted AFTER hg0's masks (off the
            # first-matmul critical path); they only gate relu (~21us),
            # w1 masks (~30us) and finalize (~45us).
            gbu = const.tile([P, 52], F32, tag="gbu")
            bb0 = const.tile([P, H // P], F32, tag="bb0")
            rec = const.tile([P, 1], F32, tag="rec")
            rz2 = const.tile([P, 1], F32, tag="rz2")
            gz = const.tile([P, 48], F32, tag="gz")
            bb1 = const.tile([P, D // P], F32, tag="bb1")

            def emit_gbu_derived():
                nc.vector.tensor_copy(gbu[:], g_ps[:])
                nc.vector.tensor_tensor(
                    bb0[:].rearrange("p (s i) -> p s i", i=2),
                    b0_sb[:].rearrange("p (s i) -> p s i", i=2),
                    gbu[:, 0:16].unsqueeze(2).to_broadcast((P, 16, 2)),
                    ALU.mult,
                )
                nc.vector.reciprocal(rec[:], gbu[:, 51:52])
                nc.vector.tensor_tensor(rz2[:], rec[:], rec[:], ALU.mult)
                nc.vector.tensor_scalar(gz[:], gbu[:, 0:48], rz2[:, 0:1], None, ALU.mult)
                for j, (c0, c1) in enumerate([(0, 4), (4, 6), (6, 8)]):
                    nc.vector.tensor_scalar(
                        bb1[:, c0:c1], b1_sb[:, c0:c1],
                        gbu[:, 48 + j:49 + j], rec[:, 0:1], ALU.mult, ALU.mult,
                    )

            # persistent hT and output accumulator
            ht_sb = [
                const.tile([P, T], BF16, tag=f"ht{m}", name=f"ht{m}")
                for m in range(H // P)
            ]
            outacc = [
                const.tile([P, T], F32, tag=f"oa{dt}", name=f"oa{dt}")
                for dt in range(D // P)
            ]

            def emit_fill(n, name, pool, lhsT=None):
                # lhsT pins a dependency so the Tile scheduler cannot hoist
                # pacing junk ahead of the mask chain (its DMA cost model
                # mispredicts arch-DMA latency and would reorder otherwise)
                if n <= 0:
                    return
                ps_f = pool.tile([P, T], F32, tag=pool is ps0_pool and "ps0" or "ps1", name=name)
                for i in range(n):
                    w = lhsT if lhsT is not None else junk_w[:, (i % 2) * P:(i % 2) * P + P]
                    nc.tensor.matmul(
                        ps_f[:], w, junk_x[:],
                        start=(i == 0), stop=(i == n - 1),
                    )

            def mask_w0(hg, pk, gsrc=None):
                """mask+cast one w0f tile -> two [P, 512] bf16 chunks (DVE)."""
                g = gsrc if gsrc is not None else gbu
                w0f = w0f_tiles[(hg, pk)]
                chunks = []
                for c in range(2):
                    cbase = _DBLK[2 * pk + c] * 16 + hg * 2
                    w0m = w0_pool.tile([P, 512], BF16, tag="w0m", name="w0m")
                    nc.vector.tensor_tensor(
                        w0m[:].rearrange("p (s c) -> p s c", c=SEG),
                        w0f[:, c * 512:(c + 1) * 512].rearrange(
                            "p (s c) -> p s c", c=SEG
                        ),
                        g[:, cbase:cbase + 2]
                        .unsqueeze(2)
                        .to_broadcast((P, 2, SEG)),
                        ALU.mult,
                    )
                    chunks.append(w0m)
                return chunks

            pending_fin = []

            def emit_l0_loads(hg):
                for pk in range(4):
                    load_w0f(hg, pk)

            def emit_l0(pr, pace=None, preloaded=False, only_hg=None):
                """L0 for h-groups 2pr, 2pr+1, k-major (4 live chains)."""
                hgs = (2 * pr, 2 * pr + 1) if only_hg is None else (only_hg,)
                for hg in hgs:
                    pc = pace.get(hg, {}) if pace else {}
                    w0m_chunks = []
                    for pk in range(4):
                        if not preloaded:
                            load_w0f(hg, pk)
                        w0m_chunks.extend(
                            mask_w0(hg, pk, gsrc=g_ps if hg == 0 else None)
                        )
                        for _ in range(2):
                            if pending_fin:
                                pending_fin.pop(0)()
                    if hg == 0:
                        emit_gbu_derived()
                    pool, ptag = (ps0_pool, "ps0") if hg % 2 == 0 else (ps1_pool, "ps1")
                    pss = [
                        pool.tile([P, T], F32, tag=ptag, name=f"ps0_{hg}_{ht}")
                        for ht in range(4)
                    ]
                    for k in range(D // P):
                        for ht in range(4):
                            nc.tensor.matmul(
                                pss[ht][:],
                                w0m_chunks[k][:, ht * P:(ht + 1) * P],
                                xt_sb[k][:],
                                start=(k == 0),
                                stop=(k == D // P - 1),
                            )
                        if k in pc:
                            emit_fill(pc[k], f"pace_{hg}_{k}", ps1_pool)
                    for ht in range(4):
                        m = hg * 4 + ht
                        if hg % 2 == 1 and ht >= 2:
                            # odd-hg tail relus on DVE: ACT's serialized
                            # drain (+2.0 us for the 4th) otherwise gates
                            # L1's dt6/dt7 PSUM banks (~432 ns per phase)
                            nc.vector.tensor_scalar(
                                ht_sb[m][:], pss[ht][:], bb0[:, m:m + 1], 0.0,
                                ALU.add, ALU.max,
                            )
                        else:
                            nc.scalar.activation(
                                ht_sb[m][:], pss[ht][:], AF.Relu, bias=bb0[:, m:m + 1]
                            )

            def emit_l1_dma(pr):
                """w1 tile DMAs for pair pr (sync queue, after w0 loads)."""
                tiles = []
                for pj in range(4):
                    hc = pr * 8 + 2 * pj
                    w1f = w1f_pool.tile([P, 2048], F32, tag="w1f", name="w1f")
                    nc.sync.dma_start(
                        w1f[:].rearrange("p (k d) -> p k d", k=2),
                        w1T_d[hc * P:(hc + 2) * P, :].rearrange(
                            "(k p) d -> p k d", k=2
                        ),
                    )
                    w1m = w1_pool.tile([P, 2048], BF16, tag="w1m", name="w1m")
                    tiles.append((w1f, w1m))
                return tiles

            def emit_w1_mask(pr, tiles, pj, eng):
                """mask+cast one w1 pair; DVE runs data-gated, ACT sections
                are placed between relu drains so they never queue late."""
                w1f, w1m = tiles[pj]
                seg_h = (pr * 8 + 2 * pj) // 2
                ap3m = w1m[:].rearrange("p (k d) -> p k d", k=2)
                ap3f = w1f[:].rearrange("p (k d) -> p k d", k=2)
                for jd, (c0, c1) in enumerate([(0, 512), (512, 768), (768, 1024)]):
                    sc = gz[:, jd * 16 + seg_h:jd * 16 + seg_h + 1]
                    if eng is nc.vector:
                        nc.vector.tensor_scalar(
                            ap3m[:, :, c0:c1], ap3f[:, :, c0:c1],
                            sc, None, ALU.mult,
                        )
                    else:
                        nc.scalar.activation(
                            ap3m[:, :, c0:c1], ap3f[:, :, c0:c1], AF.Copy,
                            scale=sc,
                        )

            def emit_l1(pr, w1m_pairs):
                """Layer 1 matmuls for pair pr (K = 8 x 128)."""
                w1m_tiles = [m for _, m in w1m_pairs]

                def finalize(dt, ps):
                    if pr == 0:
                        nc.scalar.activation(
                            outacc[dt][:], ps[:], AF.Identity, bias=bb1[:, dt:dt + 1]
                        )
                    elif pr < 3:
                        nc.vector.tensor_tensor(
                            outacc[dt][:], ps[:], outacc[dt][:], ALU.add
                        )
                    else:
                        # adds in halves (second overlaps the first), then one
                        # full-tile store on the idle sync queue
                        for c0, c1 in ((0, T // 2), (T // 2, T)):
                            nc.vector.tensor_tensor(
                                outacc[dt][:, c0:c1], ps[:, c0:c1],
                                outacc[dt][:, c0:c1], ALU.add,
                            )
                        nc.sync.dma_start(
                            out_d[dt * P:(dt + 1) * P, :], outacc[dt][:]
                        )

                if pr < 3:
                    # pj-major over all 8 output chains (all 8 PSUM banks):
                    # w1 pairs consumed in DMA arrival order, K-split deep
                    pss = [
                        (ps0_pool if dt < 4 else ps1_pool).tile(
                            [P, T], F32, tag="ps0" if dt < 4 else "ps1",
                            name=f"ps1_{pr}_{dt}"
                        )
                        for dt in range(8)
                    ]
                    for pj in range(4):
                        for j in (2 * pj, 2 * pj + 1):
                            for dt in range(8):
                                off = (j % 2) * 1024 + dt * P
                                nc.tensor.matmul(
                                    pss[dt][:],
                                    w1m_tiles[pj][:, off:off + P],
                                    ht_sb[pr * 8 + j][:],
                                    start=(j == 0),
                                    stop=(j == 7),
                                )
                    for dt in range(8):
                        pending_fin.append(
                            (lambda d=dt, p=pss[dt]: finalize(d, p))
                        )
                else:
                    # dt-major: chains end staggered so the finalize/store
                    # tail pipelines; the last dt runs as two column-half
                    # chains so its final add+store covers only 128 KB
                    for dt in range(D // P):
                        pool, ptag = (ps0_pool, "ps0") if dt < 4 else (ps1_pool, "ps1")
                        ps = pool.tile([P, T], F32, tag=ptag, name="ps1")
                        if dt < 7:
                            for j in range(8):
                                off = (j % 2) * 1024 + dt * P
                                nc.tensor.matmul(
                                    ps[:],
                                    w1m_tiles[j // 2][:, off:off + P],
                                    ht_sb[pr * 8 + j][:],
                                    start=(j == 0),
                                    stop=(j == 7),
                                )
                            finalize(dt, ps)
                        else:
                            for c0, c1, eng in ((0, T // 2, nc.sync), (T // 2, T, nc.scalar)):
                                for j in range(8):
                                    off = (j % 2) * 1024 + dt * P
                                    nc.tensor.matmul(
                                        ps[:, c0:c1],
                                        w1m_tiles[j // 2][:, off:off + P],
                                        ht_sb[pr * 8 + j][:, c0:c1],
                                        start=(j == 0),
                                        stop=(j == 7),
                                    )
                                nc.vector.tensor_tensor(
                                    outacc[dt][:, c0:c1], ps[:, c0:c1],
                                    outacc[dt][:, c0:c1], ALU.add,
                                )
                                eng.dma_start(
                                    out_d[dt * P:(dt + 1) * P, c0:c1],
                                    outacc[dt][:, c0:c1],
                                )

            # ---------------- phases ----------------
            emit_l0(0, pace={0: PACE_HG0, 1: PACE_HG1}, preloaded=True)
            tp = emit_l1_dma(0)
            for pj in range(4):
                emit_w1_mask(0, tp, pj, nc.vector)
            emit_fill(FILL2, "fill2", ps1_pool)
            emit_l1(0, tp)
            for pr in range(1, 4):
                emit_l0(pr)
                tp = emit_l1_dma(pr)
                for pj in range(4):
                    emit_w1_mask(pr, tp, pj, nc.vector)
                emit_l1(pr, tp)

    nc.compile()
    return nc


_NC_CACHE: dict[str, bass.Bass] = {}


def _get_nc() -> bass.Bass:
    key = f"{NW1}_{NW2}"
    if key not in _NC_CACHE:
        _NC_CACHE[key] = _build_nc()
    return _NC_CACHE[key]


def make_in_maps(x, w0, b0, w1, b1, arch_embed, arch_mlp):
    """Host-side layout prep (pure reshape/transpose/tile, no arithmetic)."""
    w0T = np.ascontiguousarray(w0.T)                       # [D, H]
    w1T = np.ascontiguousarray(w1.T)                       # [H, D]
    b0r = np.ascontiguousarray(b0.reshape(H // P, P).T)    # [P, 32]
    b1r = np.ascontiguousarray(b1.reshape(D // P, P).T)    # [P, 8]
    # packed [ae9 | am9 | K2]: pure repeat/tile/concat layout, no arithmetic
    arch = np.concatenate(
        [
            np.repeat(arch_embed, 3).reshape(9, 1),
            np.tile(arch_mlp, 3).reshape(9, 1),
            _K2,
        ],
        axis=1,
    ).astype(np.float32)
    arch = np.ascontiguousarray(arch)                      # [9, 54]
    x3 = x.reshape(N_CORES, T, D)
    return [
        {
            "xT": np.ascontiguousarray(x3[c].T),           # [D, T]
            "w0T": w0T,
            "w1T": w1T,
            "b0r": b0r,
            "b1r": b1r,
            "arch": arch,
        }
        for c in range(N_CORES)
    ]


def kernel(x, w0, b0, w1, b1, arch_embed, arch_mlp):
    x = np.asarray(x, dtype=np.float32)
    w0 = np.asarray(w0, dtype=np.float32)
    b0 = np.asarray(b0, dtype=np.float32)
    w1 = np.asarray(w1, dtype=np.float32)
    b1 = np.asarray(b1, dtype=np.float32)
    arch_embed = np.asarray(arch_embed, dtype=np.float32)
    arch_mlp = np.asarray(arch_mlp, dtype=np.float32)

    in_maps = make_in_maps(x, w0, b0, w1, b1, arch_embed, arch_mlp)
    nc = _get_nc()
    res = run_bass_kernel_spmd(nc, in_maps, core_ids=list(range(N_CORES)))
    out = np.stack([res.results[c]["outT"].T for c in range(N_CORES)], axis=0)
    return np.ascontiguousarray(out)  # [8, 512, 1024] float32


# revision 30
# speedup vs baseline: 1.0079x; 1.0079x over previous
"""Trainium2 Bass kernel for nn_MixedFeedFoward (DARTS-style mixed-architecture MLP).

Math: out = relu(x @ (m0*w0).T + bm0*b0) @ (m1*w1).T + bm1*b1
The DARTS masks are rank-structured.  With a = softmax(arch_embed),
b = softmax(arch_mlp), EMBED = (512,768,1024), RATIO = (2,3,4):

  s_e[h]     = sum_r b_r * [h < e*r]
  g_j[h]     = sum_{e_idx >= j} a_e * s_e[h]
  c_j        = sum_{e_idx >= j} a_e
  W0eff[h,d] = w0[h,d] * g_{blk(d)}[h]      blk(d): 0 for d<512, 1 for d<768, else 2
  bm0[h]     = g_0[h]
  W1eff[d,h] = w1[d,h] * g_{blk(d)}[h]
  bm1[d]     = c_{blk(d)}

g_j is constant on 256-aligned h segments, so masking reduces to 51 runtime
scalars computed on device from the arch inputs via one tiny matmul against a
constant 0/1 selection table (e9 broadcast to 128 columns makes the matmul
output land on all 128 partitions directly).  The softmax normalizer Z
(col 51, all-ones) is NOT applied on the critical path: w0 is masked with the
UNNORMALIZED gu = Z*g (h comes out scaled by Z), and the w1 mask scalars are
premultiplied by 1/Z^2, so L1 partials land at true scale and the tail
finalize is a plain add+store.  relu commutes with the positive Z scaling, so
numerics match the normalized form to bf16 rounding.

Sharding: data-parallel over the 4096 tokens -> 512 tokens per core; every
core streams the full weights.  PE roofline: 512 matmuls x 216 ns = 110.6 us.
The schedule is supply-shaped: measured per-core in-bound DMA is ~300 GB/s
for the first ~4 MiB (8-core HBM contention at start) and ~400 GB/s after,
while real matmuls demand 290 GB/s — so the opening is data-limited and the
binding bound is  t(first 4 MiB) + (512-32)*216ns + tail  ~=  131 us.

  - q1 (sync HWDGE) carries, in exact k-step consumption order: arch row,
    x0 x1, w0(hg0,pk0), x2 x3, w0(pk1), x4 x5, w0(pk2), x6 x7, w0(pk3),
    w0(hg1), then the steady w1/w0 phase stream.  Output stores also go on
    q1's engine (sync) — it is idle in the tail, unlike Scalar whose queue
    serializes ~0.6 us per DMA trigger behind its activation work.
  - PE warmup (pinned to scheduler priority 0 with its junk memsets so it
    starts at engine boot ~8 us every build): NW1 big junk matmuls bridge
    the HAM cold window, the tiny mask matmul runs when exp(arch) lands
    (~12 us; DMA completion sems add ~2 us over data arrival), then NW2
    64-column junk matmuls (54 ns each, so overshoot is nearly free against
    the free-running +-3.4 us HAM window phase), then real work ~14 us.
  - PACE_* junk matmuls are interleaved into hg0's k-steps, sized to the
    measured supply curve, so the PE never accumulates enough idle density
    to re-trigger the HAM half-clock window (2x cost on the next ~7 us).
Steady state: L0 k-major with 4 concurrent PSUM chains per 512-row h-group
(even h-groups on ps0 banks, odd on ps1); L1 for pr<3 runs pj-major across
ALL 8 output chains (8 PSUM banks, w1 pairs consumed in DMA arrival order);
L1(pr3) runs dt-major so chains end staggered and the store tail pipelines,
with the last dt as two column-half chains (its final store covers 128 KB).
All w0/w1 masks on DVE, emitted in arrival order (ACT masks would queue
behind relu drains, holding w1f bufs and throttling prefetch; GpSimd is 3x
too slow); L1 finalizes are DEFERRED into the next L0's mask emission so
mask production never queues behind 8 back-to-back PSUM-drain adds, and
each freed ps bank is recycled just-in-time.  pr0 finalize (copy+bias) runs
on ACT via AF.Identity.

Remaining known overheads (measured): ~8 us engine boot, ~4 us HAM cold
ramp, ~6 us data-bound opening slack, ~10x ~432 ns periodic PE stalls
(LDWEIGHTS/start-matmuls waiting on just-in-time DVE mask production and
PSUM-bank drain sems -- rebalancing engines (ACT/GpSimd masks, DVE relus)
was tried repeatedly and always shifted, never removed, these waits; the
Tile scheduler's static schedule also swings +-2 us between builds), and
~6 us tail (2 us last add+store+flight+receipt, ~3.5 us NEFF drain
protocol).  fp8 DoubleRow (2x PE) was measured working at 216 ns/instr
but is numerically infeasible: e4m3 quantization is 2.65% RMS per tensor
-> >=3.7% output error vs the 2e-2 gate (bf16 gives 3.3e-3).  Multi-queue
DMA does NOT raise the ~390 GB/s per-core in-bound cap (HBM-per-NC
limit).  Final opening: hg0's mask ops read the mask-scalar PSUM directly
(the gbu copy + derived scalars are emitted after hg0's masks, off the
first-matmul critical path), and NW2 is sized for the COLD tiny-junk rate
so a late HAM fire cannot push real work past data-readiness.  Do NOT
shrink the xf staging tiles into a reused ring: a staging-slot reuse makes
an x-load DMA trigger wait on a cast, and since the sync queue is in-order
that stalls the entire weight stream behind it (measured 165 us).  SBUF is
full: w0f=10 overflows by ~4 KB/partition.  Measured 136.6-138.0 us over
8 runs, one 163.7 us outlier from transient machine noise (was
142.5-144.4).
"""

import os

import numpy as np

import concourse.bass as bass
import concourse.mybir as mybir
from concourse import bacc
from concourse.bass_utils import run_bass_kernel_spmd
from concourse.tile import TileContext

N_CORES = 8
D = 1024          # embed dim
H = 4096          # expansion dim
T = 512           # tokens per core (4096 total / 8 cores)
P = 128
SEG = 256         # h-segment size on which g_j is constant
NSEG = H // SEG   # 16
EMBED = (512, 768, 1024)
RATIO = (2, 3, 4)

F32 = mybir.dt.float32
BF16 = mybir.dt.bfloat16
AF = mybir.ActivationFunctionType
ALU = mybir.AluOpType

NW1 = int(os.environ.get("BASS_NW1", "13"))     # junk MMs before the mask matmul
NW2 = int(os.environ.get("BASS_NW2", "16"))    # junk MMs after it, before real work
# pacing junk inside the supply-starved opening: {after_kstep: count}
PACE_HG0 = {1: int(os.environ.get("BASS_PH0A", "8")),
            3: int(os.environ.get("BASS_PH0B", "3")),
            5: int(os.environ.get("BASS_PH0C", "5"))}
PACE_HG1 = {3: int(os.environ.get("BASS_PH1A", "0"))}
FILL2 = int(os.environ.get("BASS_FILL2", "0"))


def _build_k2() -> np.ndarray:
    """Constant 0/1 selection table: G_flat[col] = sum_i E9[i] * K2[i, col]
    where E9[e*3+r] = exp(ae[e] + am[r]).
    cols 0..47: col = j*16 + seg -> [e_idx >= j] * [seg*SEG < e*r]
    cols 48..50: col = 48 + j   -> [e_idx >= j]   (since sum_r b_r = 1)
    col 51: all ones -> sum(E9), the softmax normalizer Z
    """
    k2 = np.zeros((9, 52), dtype=np.float32)
    for ie, e in enumerate(EMBED):
        for ir, r in enumerate(RATIO):
            i = ie * 3 + ir
            for j in range(3):
                if ie >= j:
                    for seg in range(NSEG):
                        if seg * SEG < e * r:
                            k2[i, j * 16 + seg] = 1.0
                    k2[i, 48 + j] = 1.0
            k2[i, 51] = 1.0
    return k2


_K2 = _build_k2()

# d-block of each 128-wide d-chunk (0..7): [0,512)->0, [512,768)->1, [768,1024)->2
_DBLK = [0, 0, 0, 0, 1, 1, 2, 2]


def _build_nc() -> bass.Bass:
    nc = bacc.Bacc("TRN2", target_bir_lowering=False, debug=False)

    xT_d = nc.dram_tensor("xT", [D, T], F32, kind="ExternalInput")
    w0T_d = nc.dram_tensor("w0T", [D, H], F32, kind="ExternalInput")
    w1T_d = nc.dram_tensor("w1T", [H, D], F32, kind="ExternalInput")
    b0r_d = nc.dram_tensor("b0r", [P, H // P], F32, kind="ExternalInput")
    b1r_d = nc.dram_tensor("b1r", [P, D // P], F32, kind="ExternalInput")
    # arch = [ae9 | am9 | K2] packed in one tensor: a single 216B-row DMA
    arch_d = nc.dram_tensor("arch", [9, 54], F32, kind="ExternalInput")
    out_d = nc.dram_tensor("outT", [D, T], F32, kind="ExternalOutput")

    with TileContext(nc) as tc:
        with (
            tc.tile_pool(name="const", bufs=1) as const,
            tc.tile_pool(name="w0f", bufs=8) as w0f_pool,
            tc.tile_pool(name="xfp", bufs=1) as xf_pool,
            tc.tile_pool(name="w0p", bufs=16) as w0_pool,
            tc.tile_pool(name="w1f", bufs=7) as w1f_pool,
            tc.tile_pool(name="w1p", bufs=6) as w1_pool,
            tc.tile_pool(name="ps0", bufs=4, space="PSUM") as ps0_pool,
            tc.tile_pool(name="ps1", bufs=4, space="PSUM") as ps1_pool,
        ):
            # ---------------- tiny constants (no deps; DVE runs them first) --
            # junk memsets + warmup pinned to scheduler priority 0: the PE
            # warmup must start right at engine boot (~7.8 us) on every build
            junk_w = const.tile([P, 2 * P], BF16, tag="junk_w")
            junk_x = const.tile([P, T], BF16, tag="junk_x")
            with tc.high_priority():
                nc.vector.memset(junk_w[:], 0.0)
                nc.vector.memset(junk_x[:], 0.0)
                ps_w = ps0_pool.tile([P, T], F32, tag="ps0", name="ps_w")
                for i in range(NW1):
                    sl = (i % 2) * P
                    nc.tensor.matmul(
                        ps_w[:], junk_w[:, sl:sl + P], junk_x[:],
                        start=(i == 0), stop=(i == NW1 - 1),
                    )
            ps_w = ps0_pool.tile([P, T], F32, tag="ps0", name="ps_w")
            for i in range(NW1):
                sl = (i % 2) * P
                nc.tensor.matmul(
                    ps_w[:], junk_w[:, sl:sl + P], junk_x[:],
                    start=(i == 0), stop=(i == NW1 - 1),
                )

            ones9 = const.tile([9, P], F32, tag="ones9")
            nc.vector.memset(ones9[:], 1.0)

            # ---------------- arch-weight prep (q1 first: tiny, low latency) -
            arch_sb = const.tile([9, 54], F32, tag="arch_sb")
            nc.sync.dma_start(arch_sb[:], arch_d[:, :])
            k2_sb = arch_sb[:, 2:54]

            # e9 = exp(ae + am) in one ACT op; broadcast along free dim on DVE
            e9 = const.tile([9, 1], F32, tag="e9")
            nc.scalar.activation(e9[:], arch_sb[:, 0:1], AF.Exp, bias=arch_sb[:, 1:2])
            e9r = const.tile([9, P], F32, tag="e9r")
            nc.vector.tensor_scalar(e9r[:], ones9[:], e9[:, 0:1], None, ALU.mult)

            # biases ride the scalar HWDGE queue (triggers issue before exp)
            b0_sb = const.tile([P, H // P], F32, tag="b0_sb")
            nc.scalar.dma_start(b0_sb[:], b0r_d[:, :])
            b1_sb = const.tile([P, D // P], F32, tag="b1_sb")
            nc.scalar.dma_start(b1_sb[:], b1r_d[:, :])

            # ---------------- x + w0(hg0/hg1) on q1 in consumption order -----
            xfs = []
            xt_sb = []
            for k in range(D // P):
                xf = xf_pool.tile([P, T], F32, tag=f"xf{k}", name=f"xf{k}", bufs=1)
                t = const.tile([P, T], BF16, tag=f"xt{k}", name=f"xt{k}")
                xfs.append(xf)
                xt_sb.append(t)

            def load_x(k):
                nc.sync.dma_start(xfs[k][:], xT_d[k * P:(k + 1) * P, :])

            w0f_tiles = {}  # (hg, pk) -> tile

            def load_w0f(hg, pk):
                w0f = w0f_pool.tile([P, 1024], F32, tag="w0f", name="w0f")
                w0f_tiles[(hg, pk)] = w0f
                nc.sync.dma_start(
                    w0f[:].rearrange("p (k h) -> p k h", k=2),
                    w0T_d[
                        2 * pk * P:(2 * pk + 2) * P,
                        hg * 512:(hg + 1) * 512,
                    ].rearrange("(k p) h -> p k h", k=2),
                )

            # q1 order = k-step consumption order of L0(hg0)
            load_x(0)
            load_x(1)
            load_w0f(0, 0)
            load_x(2)
            load_x(3)
            load_w0f(0, 1)
            load_x(4)
            load_x(5)
            load_w0f(0, 2)
            load_x(6)
            load_x(7)
            load_w0f(0, 3)
            for pk in range(4):
                load_w0f(1, pk)

            # x casts: first two chunks on DVE (needed by the first k-steps),
            # the rest on Scalar (queued after exp)
            for k in range(2):
                nc.vector.tensor_copy(xt_sb[k][:], xfs[k][:])
            for k in range(2, D // P):
                nc.scalar.activation(xt_sb[k][:], xfs[k][:], AF.Copy)

            # ---------------- PE: mask matmul, then junk until data -------
            g_ps = ps1_pool.tile([P, T], F32, tag="ps1", name="g_ps")[:, 0:52]
            nc.tensor.matmul(g_ps[:], e9r[:], k2_sb[:], start=True, stop=True)
            ps_w2 = ps0_pool.tile([P, T], F32, tag="ps0", name="ps_w2")
            for i in range(NW2):
                sl = (i % 2) * P
                nc.tensor.matmul(
                    ps_w2[:, 0:64], junk_w[:, sl:sl + P], junk_x[:, 0:64],
                    start=(i == 0), stop=(i == NW2 - 1),
                )

            # gbu = unnormalized mask scalars [128, 52]; col 51 = Z.
            # hg0's mask ops read g_ps (PSUM) directly -- the copy and all
            # derived scalars are emitted AFTER hg0's masks (off the
            # first-matmul critical path); they only gate relu (~21us),
            # w1 masks (~30us) and finalize (~45us).
            gbu = const.tile([P, 52], F32, tag="gbu")
            bb0 = const.tile([P, H // P], F32, tag="bb0")
            rec = const.tile([P, 1], F32, tag="rec")
            rz2 = const.tile([P, 1], F32, tag="rz2")
            gz = const.tile([P, 48], F32, tag="gz")
            bb1 = const.tile([P, D // P], F32, tag="bb1")

            def emit_gbu_derived():
                nc.vector.tensor_copy(gbu[:], g_ps[:])
                nc.vector.tensor_tensor(
                    bb0[:].rearrange("p (s i) -> p s i", i=2),
                    b0_sb[:].rearrange("p (s i) -> p s i", i=2),
                    gbu[:, 0:16].unsqueeze(2).to_broadcast((P, 16, 2)),
                    ALU.mult,
                )
                nc.vector.reciprocal(rec[:], gbu[:, 51:52])
                nc.vector.tensor_tensor(rz2[:], rec[:], rec[:], ALU.mult)
                nc.vector.tensor_scalar(gz[:], gbu[:, 0:48], rz2[:, 0:1], None, ALU.mult)
                for j, (c0, c1) in enumerate([(0, 4), (4, 6), (6, 8)]):
                    nc.vector.tensor_scalar(
                        bb1[:, c0:c1], b1_sb[:, c0:c1],
                        gbu[:, 48 + j:49 + j], rec[:, 0:1], ALU.mult, ALU.mult,
                    )

            # persistent hT and output accumulator
            ht_sb = [
                const.tile([P, T], BF16, tag=f"ht{m}", name=f"ht{m}")
                for m in range(H // P)
            ]
            outacc = [
                const.tile([P, T], F32, tag=f"oa{dt}", name=f"oa{dt}")
                for dt in range(D // P)
            ]

            def emit_fill(n, name, pool, lhsT=None):
                # lhsT pins a dependency so the Tile scheduler cannot hoist
                # pacing junk ahead of the mask chain (its DMA cost model
                # mispredicts arch-DMA latency and would reorder otherwise)
                if n <= 0:
                    return
                ps_f = pool.tile([P, T], F32, tag=pool is ps0_pool and "ps0" or "ps1", name=name)
                for i in range(n):
                    w = lhsT if lhsT is not None else junk_w[:, (i % 2) * P:(i % 2) * P + P]
                    nc.tensor.matmul(
                        ps_f[:], w, junk_x[:],
                        start=(i == 0), stop=(i == n - 1),
                    )

            def mask_w0(hg, pk, gsrc=None):
                """mask+cast one w0f tile -> two [P, 512] bf16 chunks (DVE)."""
                g = gsrc if gsrc is not None else gbu
                w0f = w0f_tiles[(hg, pk)]
                chunks = []
                for c in range(2):
                    cbase = _DBLK[2 * pk + c] * 16 + hg * 2
                    w0m = w0_pool.tile([P, 512], BF16, tag="w0m", name="w0m")
                    nc.vector.tensor_tensor(
                        w0m[:].rearrange("p (s c) -> p s c", c=SEG),
                        w0f[:, c * 512:(c + 1) * 512].rearrange(
                            "p (s c) -> p s c", c=SEG
                        ),
                        g[:, cbase:cbase + 2]
                        .unsqueeze(2)
                        .to_broadcast((P, 2, SEG)),
                        ALU.mult,
                    )
                    chunks.append(w0m)
                return chunks

            pending_fin = []

            def emit_l0_loads(hg):
                for pk in range(4):
                    load_w0f(hg, pk)

            def emit_l0(pr, pace=None, preloaded=False, only_hg=None):
                """L0 for h-groups 2pr, 2pr+1, k-major (4 live chains)."""
                hgs = (2 * pr, 2 * pr + 1) if only_hg is None else (only_hg,)
                for hg in hgs:
                    pc = pace.get(hg, {}) if pace else {}
                    w0m_chunks = []
                    for pk in range(4):
                        if not preloaded:
                            load_w0f(hg, pk)
                        w0m_chunks.extend(
                            mask_w0(hg, pk, gsrc=g_ps if hg == 0 else None)
                        )
                        for _ in range(2):
                            if pending_fin:
                                pending_fin.pop(0)()
                    if hg == 0:
                        emit_gbu_derived()
                    pool, ptag = (ps0_pool, "ps0") if hg % 2 == 0 else (ps1_pool, "ps1")
                    pss = [
                        pool.tile([P, T], F32, tag=ptag, name=f"ps0_{hg}_{ht}")
                        for ht in range(4)
                    ]
                    for k in range(D // P):
                        for ht in range(4):
                            nc.tensor.matmul(
                                pss[ht][:],
                                w0m_chunks[k][:, ht * P:(ht + 1) * P],
                                xt_sb[k][:],
                                start=(k == 0),
                                stop=(k == D // P - 1),
                            )
                        if k in pc:
                            emit_fill(pc[k], f"pace_{hg}_{k}", ps1_pool)
                    for ht in range(4):
                        m = hg * 4 + ht
                        nc.scalar.activation(
                            ht_sb[m][:], pss[ht][:], AF.Relu, bias=bb0[:, m:m + 1]
                        )

            def emit_l1_dma(pr):
                """w1 tile DMAs for pair pr (sync queue, after w0 loads)."""
                tiles = []
                for pj in range(4):
                    hc = pr * 8 + 2 * pj
                    w1f = w1f_pool.tile([P, 2048], F32, tag="w1f", name="w1f")
                    nc.sync.dma_start(
                        w1f[:].rearrange("p (k d) -> p k d", k=2),
                        w1T_d[hc * P:(hc + 2) * P, :].rearrange(
                            "(k p) d -> p k d", k=2
                        ),
                    )
                    w1m = w1_pool.tile([P, 2048], BF16, tag="w1m", name="w1m")
                    tiles.append((w1f, w1m))
                return tiles

            def emit_w1_mask(pr, tiles, pj, eng):
                """mask+cast one w1 pair; DVE runs data-gated, ACT sections
                are placed between relu drains so they never queue late."""
                w1f, w1m = tiles[pj]
                seg_h = (pr * 8 + 2 * pj) // 2
                ap3m = w1m[:].rearrange("p (k d) -> p k d", k=2)
                ap3f = w1f[:].rearrange("p (k d) -> p k d", k=2)
                for jd, (c0, c1) in enumerate([(0, 512), (512, 768), (768, 1024)]):
                    sc = gz[:, jd * 16 + seg_h:jd * 16 + seg_h + 1]
                    if eng is nc.vector:
                        nc.vector.tensor_scalar(
                            ap3m[:, :, c0:c1], ap3f[:, :, c0:c1],
                            sc, None, ALU.mult,
                        )
                    else:
                        nc.scalar.activation(
                            ap3m[:, :, c0:c1], ap3f[:, :, c0:c1], AF.Copy,
                            scale=sc,
                        )

            def emit_l1(pr, w1m_pairs):
                """Layer 1 matmuls for pair pr (K = 8 x 128)."""
                w1m_tiles = [m for _, m in w1m_pairs]

                def finalize(dt, ps):
                    if pr == 0:
                        nc.scalar.activation(
                            outacc[dt][:], ps[:], AF.Identity, bias=bb1[:, dt:dt + 1]
                        )
                    elif pr < 3:
                        nc.vector.tensor_tensor(
                            outacc[dt][:], ps[:], outacc[dt][:], ALU.add
                        )
                    else:
                        # adds in halves (second overlaps the first), then one
                        # full-tile store on the idle sync queue
                        for c0, c1 in ((0, T // 2), (T // 2, T)):
                            nc.vector.tensor_tensor(
                                outacc[dt][:, c0:c1], ps[:, c0:c1],
                                outacc[dt][:, c0:c1], ALU.add,
                            )
                        nc.sync.dma_start(
                            out_d[dt * P:(dt + 1) * P, :], outacc[dt][:]
                        )

                if pr < 3:
                    # pj-major over all 8 output chains (all 8 PSUM banks):
                    # w1 pairs consumed in DMA arrival order, K-split deep
                    pss = [
                        (ps0_pool if dt < 4 else ps1_pool).tile(
                            [P, T], F32, tag="ps0" if dt < 4 else "ps1",
                            name=f"ps1_{pr}_{dt}"
                        )
                        for dt in range(8)
                    ]
                    for pj in range(4):
                        for j in (2 * pj, 2 * pj + 1):
                            for dt in range(8):
                                off = (j % 2) * 1024 + dt * P
                                nc.tensor.matmul(
                                    pss[dt][:],
                                    w1m_tiles[pj][:, off:off + P],
                                    ht_sb[pr * 8 + j][:],
                                    start=(j == 0),
                                    stop=(j == 7),
                                )
                    for dt in range(8):
                        pending_fin.append(
                            (lambda d=dt, p=pss[dt]: finalize(d, p))
                        )
                else:
                    # dt-major: chains end staggered so the finalize/store
                    # tail pipelines; the last dt runs as two column-half
                    # chains so its final add+store covers only 128 KB
                    for dt in range(D // P):
                        pool, ptag = (ps0_pool, "ps0") if dt < 4 else (ps1_pool, "ps1")
                        ps = pool.tile([P, T], F32, tag=ptag, name="ps1")
                        if dt < 7:
                            for j in range(8):
                                off = (j % 2) * 1024 + dt * P
                                nc.tensor.matmul(
                                    ps[:],
                                    w1m_tiles[j // 2][:, off:off + P],
                                    ht_sb[pr * 8 + j][:],
                                    start=(j == 0),
                                    stop=(j == 7),
                                )
                            finalize(dt, ps)
                        else:
                            for c0, c1, eng in ((0, T // 2, nc.sync), (T // 2, T, nc.scalar)):
                                for j in range(8):
                                    off = (j % 2) * 1024 + dt * P
                                    nc.tensor.matmul(
                                        ps[:, c0:c1],
                                        w1m_tiles[j // 2][:, off:off + P],
                                        ht_sb[pr * 8 + j][:, c0:c1],
                                        start=(j == 0),
                                        stop=(j == 7),
                                    )
                                nc.vector.tensor_tensor(
                                    outacc[dt][:, c0:c1], ps[:, c0:c1],
                                    outacc[dt][:, c0:c1], ALU.add,
                                )
                                eng.dma_start(
                                    out_d[dt * P:(dt + 1) * P, c0:c1],
                                    outacc[dt][:, c0:c1],
                                )

            # ---------------- phases ----------------
            emit_l0(0, pace={0: PACE_HG0, 1: PACE_HG1}, preloaded=True)
            tp = emit_l1_dma(0)
            for pj in range(4):
                emit_w1_mask(0, tp, pj, nc.vector)
            emit_fill(FILL2, "fill2", ps1_pool)
            emit_l1(0, tp)
            for pr in range(1, 4):
                emit_l0(pr)
                tp = emit_l1_dma(pr)
                for pj in range(4):
                    emit_w1_mask(pr, tp, pj, nc.vector)
                emit_l1(pr, tp)

    nc.compile()
    return nc


_NC_CACHE: dict[str, bass.Bass] = {}


def _get_nc() -> bass.Bass:
    key = f"{NW1}_{NW2}"
    if key not in _NC_CACHE:
        _NC_CACHE[key] = _build_nc()
    return _NC_CACHE[key]


def make_in_maps(x, w0, b0, w1, b1, arch_embed, arch_mlp):
    """Host-side layout prep (pure reshape/transpose/tile, no arithmetic)."""
    w0T = np.ascontiguousarray(w0.T)                       # [D, H]
    w1T = np.ascontiguousarray(w1.T)                       # [H, D]
    b0r = np.ascontiguousarray(b0.reshape(H // P, P).T)    # [P, 32]
    b1r = np.ascontiguousarray(b1.reshape(D // P, P).T)    # [P, 8]
    # packed [ae9 | am9 | K2]: pure repeat/tile/concat layout, no arithmetic
    arch = np.concatenate(
        [
            np.repeat(arch_embed, 3).reshape(9, 1),
            np.tile(arch_mlp, 3).reshape(9, 1),
            _K2,
        ],
        axis=1,
    ).astype(np.float32)
    arch = np.ascontiguousarray(arch)                      # [9, 54]
    x3 = x.reshape(N_CORES, T, D)
    return [
        {
            "xT": np.ascontiguousarray(x3[c].T),           # [D, T]
            "w0T": w0T,
            "w1T": w1T,
            "b0r": b0r,
            "b1r": b1r,
            "arch": arch,
        }
        for c in range(N_CORES)
    ]


def kernel(x, w0, b0, w1, b1, arch_embed, arch_mlp):
    x = np.asarray(x, dtype=np.float32)
    w0 = np.asarray(w0, dtype=np.float32)
    b0 = np.asarray(b0, dtype=np.float32)
    w1 = np.asarray(w1, dtype=np.float32)
    b1 = np.asarray(b1, dtype=np.float32)
    arch_embed = np.asarray(arch_embed, dtype=np.float32)
    arch_mlp = np.asarray(arch_mlp, dtype=np.float32)

    in_maps = make_in_maps(x, w0, b0, w1, b1, arch_embed, arch_mlp)
    nc = _get_nc()
    res = run_bass_kernel_spmd(nc, in_maps, core_ids=list(range(N_CORES)))
    out = np.stack([res.results[c]["outT"].T for c in range(N_CORES)], axis=0)
    return np.ascontiguousarray(out)  # [8, 512, 1024] float32
